# revision 1
# baseline (speedup 1.0000x reference)
"""Trainium2 Bass kernel for nn_Attention_60567628808865.

Dense transformer attention block (B=4, S=1024, H=4096, NH=32, D=128):
  qkv = x @ W_pack; RoPE(q, k); causal-masked softmax attention; out @ W_o.

Sharding: tensor-parallel over heads across 8 NeuronCores. Each core computes
4 heads end-to-end (QKV projection with its W_pack column slice, attention,
and its W_o row-slice partial of the output projection); the host sums the 8
partial outputs.

All matmuls run in float32r (TF32) at full PE rate; accumulation is fp32 in
PSUM. Everything on-chip works in a transposed layout (features on the
partition axis) so no transposes are needed anywhere:
  qT/kT [d, t] <- lhsT=W_qk, rhs=xT      scoresT [tk, tq] <- lhsT=kT, rhs=qT
  v [t, d]     <- lhsT=xT,   rhs=W_v     attnT [d, tq]    <- lhsT=v,  rhs=expT
  out [t, f]   <- lhsT=attnT, rhs=W_o
Softmax runs unnormalized (no max-subtraction; scores are O(1) by
construction and exp(-1e9)=0), with the denominator computed by a ones-vector
matmul accumulated in PSUM and applied after PV via a K=1 broadcast matmul.
RoPE's rotate-half is a partition shift, done for free in the DMA that loads
q/k back from scratch, with the sign folded into the host-built sin table.
DMAs are batched into multi-dim-AP transfers (the HWDGE issue path costs
~625ns per DMA instruction, so many small DMAs throttle the PE).
"""
import numpy as np

import concourse.bass as bass  # noqa: F401  (AP types come via tile/bacc)
import concourse.tile as tile
from contextlib import ExitStack
from concourse import bacc, mybir
from concourse import bass_utils

F32 = mybir.dt.float32
F32R = mybir.dt.float32r
BF16 = mybir.dt.bfloat16
AF = mybir.ActivationFunctionType
ALU = mybir.AluOpType

B, S, H, NH = 4, 1024, 4096, 32
D = H // NH          # 128
T = B * S            # 4096 tokens
N_CORES = 8
HC = NH // N_CORES   # 4 heads per core
SCALE = float(1.0 / np.sqrt(D))
ROPE_BASE = 10000.0

TB = 256             # phase-1 token block (moving dim; >=256 keeps f32r at full rate)
NTB = T // TB        # 16
KT = H // 128        # 32 contraction tiles

_CACHE = {}


def _build_module(phases=("p1", "p2")):
    nc = bacc.Bacc("TRN2", target_bir_lowering=False, debug=False,
                   num_devices=N_CORES)

    xT = nc.dram_tensor("xT", [H, T], F32R, kind="ExternalInput").ap()
    wqk = nc.dram_tensor("wqk", [H, 2 * HC * D], F32R, kind="ExternalInput").ap()
    wv = nc.dram_tensor("wv", [H, HC * D], F32R, kind="ExternalInput").ap()
    wo = nc.dram_tensor("wo", [HC * D, H], F32R, kind="ExternalInput").ap()
    cosT = nc.dram_tensor("cosT", [D, T], F32, kind="ExternalInput").ap()
    sinS = nc.dram_tensor("sinS", [D, T], F32, kind="ExternalInput").ap()
    maskT = nc.dram_tensor("maskT", [B, S, S], BF16, kind="ExternalInput").ap()
    out_p = nc.dram_tensor("out_p", [T, H], F32, kind="ExternalOutput").ap()
    ones128 = nc.inline_tensor(np.ones((128, 1), np.float32), "ones128").ap().bitcast(F32R)
    ones1 = nc.inline_tensor(np.ones((1, 128), np.float32), "ones1").ap().bitcast(F32R)

    with tile.TileContext(nc) as tc, \
         nc.allow_low_precision(reason="tf32 matmuls; verified against reference"):
        with ExitStack() as octx:
            dram = octx.enter_context(tc.tile_pool(name="dram", bufs=1, space="DRAM"))
            cpool = octx.enter_context(tc.tile_pool(name="consts", bufs=1))
            # scratch: qkT rows (pass p, m): [q_2p, k_2p, q_2p+1, k_2p+1]
            qkT_d = dram.tile([2 * HC * D, T], F32R)
            v_d = dram.tile([T, HC * D], F32R)

            o128 = cpool.tile([128, 1], F32R)
            nc.sync.dma_start(o128[:], ones128[:])
            o1 = cpool.tile([1, 128], F32R)
            nc.sync.dma_start(o1[:], ones1[:])

            # ---------------- Phase 1: QKV projection ----------------
            if "p1" in phases:
              with ExitStack() as ctx:
                wpool = ctx.enter_context(tc.tile_pool(name="p1w", bufs=1))
                xpool = ctx.enter_context(tc.tile_pool(name="p1x", bufs=2))
                opool = ctx.enter_context(tc.tile_pool(name="p1o", bufs=2))
                cpool1 = ctx.enter_context(tc.tile_pool(name="p1cs", bufs=2))
                rpool1 = ctx.enter_context(tc.tile_pool(name="p1rope", bufs=2))
                pqk = ctx.enter_context(tc.tile_pool(name="p1pqk", bufs=6, space="PSUM"))
                pv = ctx.enter_context(tc.tile_pool(name="p1pv", bufs=2, space="PSUM"))

                last_x = [None]

                def load_tb_inputs(p, tb):
                    t0 = tb * TB
                    cos_tb = cpool1.tile([128, TB], F32, tag="cos")
                    nc.sync.dma_start(cos_tb[:], cosT[:, t0:t0 + TB])
                    sin_tb = cpool1.tile([128, TB], F32, tag="sin")
                    nc.sync.dma_start(sin_tb[:], sinS[:, t0:t0 + TB])
                    if p == 1 and tb == NTB - 1 and last_x[0] is not None:
                        xall = last_x[0]
                    else:
                        xall = xpool.tile([128, KT * TB], F32R, tag="x")
                        for kh in range(2):
                            nc.sync.dma_start(
                                xall[:, kh * 16 * TB:(kh + 1) * 16 * TB]
                                    .rearrange("p (kk t) -> p kk t", kk=16),
                                xT[kh * 2048:(kh + 1) * 2048, t0:t0 + TB]
                                    .rearrange("(kk p) t -> p kk t", p=128))
                    if p == 0 and tb == NTB - 1:
                        last_x[0] = xall
                    return cos_tb, sin_tb, xall

                for p in range(2):
                    # resident weights, split into half-K DMAs so the first
                    # accumulation chains can start at half-load
                    first_inputs = load_tb_inputs(p, 0 if p == 0 else NTB - 1)
                    # weights as independent half-K tiles: pass p+1's low half
                    # can reload while pass p still reads the high half
                    KH = KT // 2
                    wqk_lo = wpool.tile([128, KH * 512], F32R, tag="wqk_lo")
                    wqk_hi = wpool.tile([128, KH * 512], F32R, tag="wqk_hi")
                    wv_lo = wpool.tile([128, KH * 256], F32R, tag="wv_lo")
                    wv_hi = wpool.tile([128, KH * 256], F32R, tag="wv_hi")
                    for kh, wt in ((0, wqk_lo), (1, wqk_hi)):
                        nc.sync.dma_start(
                            wt[:].rearrange("p (kk f) -> p kk f", kk=KH),
                            wqk[kh * KH * 128:(kh + 1) * KH * 128,
                                p * 512:(p + 1) * 512]
                                .rearrange("(kk p) f -> p kk f", p=128))
                    for kh, wt in ((0, wv_lo), (1, wv_hi)):
                        nc.sync.dma_start(
                            wt[:].rearrange("p (kk f) -> p kk f", kk=KH),
                            wv[kh * KH * 128:(kh + 1) * KH * 128,
                               p * 256:(p + 1) * 256]
                                .rearrange("(kk p) f -> p kk f", p=128))

                    def wqk_sl(kk, c0, c1):
                        wt = wqk_lo if kk < KH else wqk_hi
                        return wt[:, (kk % KH) * 512 + c0:(kk % KH) * 512 + c1]

                    def wv_sl(kk):
                        wt = wv_lo if kk < KH else wv_hi
                        return wt[:, (kk % KH) * 256:(kk % KH + 1) * 256]

                    tb_order = list(range(NTB)) if p == 0 else \
                        list(reversed(range(NTB)))
                    for tb in tb_order:
                        t0 = tb * TB
                        if tb == tb_order[0]:
                            cos_tb, sin_tb, xall = first_inputs
                        else:
                            cos_tb, sin_tb, xall = load_tb_inputs(p, tb)

                        qs_all = opool.tile([128, 4 * TB], F32R, tag="qs")
                        for m in range(4):
                            ps = pqk.tile([128, TB], F32, tag="qk")
                            for kk in range(KT):
                                nc.tensor.matmul(
                                    ps[:],
                                    wqk_sl(kk, m * 128, (m + 1) * 128),
                                    xall[:, kk * TB:(kk + 1) * TB],
                                    start=(kk == 0), stop=(kk == KT - 1))
                            # RoPE fused into the epilogue: rotate-half via
                            # partition-shifted copies, sign folded into sinS
                            rot = rpool1.tile([128, TB], F32, tag="rot")
                            nc.vector.tensor_copy(rot[0:64, :], ps[64:128, :])
                            nc.vector.tensor_copy(rot[64:128, :], ps[0:64, :])
                            m1_ = rpool1.tile([128, TB], F32, tag="m1")
                            nc.vector.tensor_tensor(m1_[:], ps[:], cos_tb[:],
                                                    op=ALU.mult)
                            m2_ = rpool1.tile([128, TB], F32, tag="m2")
                            nc.vector.tensor_tensor(m2_[:], rot[:], sin_tb[:],
                                                    op=ALU.mult)
                            nc.vector.tensor_tensor(qs_all[:, m * TB:(m + 1) * TB],
                                                    m1_[:], m2_[:], op=ALU.add)
                        # one DMA: rows (p*4+m)*128 for m in 0..4
                        nc.sync.dma_start(
                            qkT_d[p * 512:(p + 1) * 512, t0:t0 + TB]
                                .rearrange("(m pp) t -> pp m t", pp=128),
                            qs_all[:].rearrange("pp (m t) -> pp m t", m=4))

                        vs_all = opool.tile([128, 2 * 256], F32R, tag="vs")
                        for mt in range(2):
                            ps = pv.tile([128, 256], F32, tag="v")
                            for kk in range(KT):
                                nc.tensor.matmul(
                                    ps[:],
                                    xall[:, kk * TB + mt * 128:kk * TB + (mt + 1) * 128],
                                    wv_sl(kk),
                                    start=(kk == 0), stop=(kk == KT - 1))
                            nc.vector.tensor_copy(vs_all[:, mt * 256:(mt + 1) * 256], ps[:])
                        nc.sync.dma_start(
                            v_d[t0:t0 + TB, p * 256:(p + 1) * 256]
                                .rearrange("(mt pp) f -> pp mt f", pp=128),
                            vs_all[:].rearrange("pp (mt f) -> pp mt f", mt=2))

            # ---------------- Phase 2+3: attention + W_o ----------------
            if "p2" in phases:
              with ExitStack() as ctx:
                wopool = ctx.enter_context(tc.tile_pool(name="p2wo", bufs=1))
                mpool = ctx.enter_context(tc.tile_pool(name="p2m", bufs=1))
                m2pool = ctx.enter_context(tc.tile_pool(name="p2m2", bufs=2))
                tpool = ctx.enter_context(tc.tile_pool(name="p2t", bufs=2))
                epool = ctx.enter_context(tc.tile_pool(name="p2e", bufs=5))
                efpool = ctx.enter_context(tc.tile_pool(name="p2ef", bufs=10))
                apool = ctx.enter_context(tc.tile_pool(name="p2a", bufs=2))
                opool = ctx.enter_context(tc.tile_pool(name="p2o", bufs=3))
                ps_s = ctx.enter_context(tc.tile_pool(name="p2ps", bufs=3, space="PSUM"))
                ps_o = ctx.enter_context(tc.tile_pool(name="p2po", bufs=2, space="PSUM"))
                ps_d = ctx.enter_context(tc.tile_pool(name="p2pd", bufs=1, space="PSUM"))
                ps_av = ctx.enter_context(tc.tile_pool(name="p2pav", bufs=2, space="PSUM"))

                # W_o resident: one tile; DMA issued after the first head's
                # input loads so attention starts immediately
                wo_a = wopool.tile([128, HC * H], F32R, tag="wo")

                for b in range(B):
                    bs = b * S
                    mask_state = [None]

                    def load_mask():
                        mask_a = m2pool.tile([128, 4 * S], BF16, tag="maskA")
                        nc.sync.dma_start(
                            mask_a[:].rearrange("p (mt t) -> p mt t", mt=4),
                            maskT[b, 0:512].rearrange("(mt p) t -> p mt t", p=128))
                        mask_bb = mpool.tile([128, 4 * S], BF16, tag="maskB")
                        nc.sync.dma_start(
                            mask_bb[:].rearrange("p (mt t) -> p mt t", mt=4),
                            maskT[b, 512:1024].rearrange("(mt p) t -> p mt t", p=128))
                        mask_state[0] = (mask_a, mask_bb)

                    if b > 0:
                        load_mask()
                    attn_t = []
                    for l in range(HC):
                        rq = (4 * (l // 2) + 2 * (l % 2)) * 128
                        vcol = (l // 2) * 256 + (l % 2) * 128

                        # rope'd q,k load: [p, j(q/k), t] (1 DMA)
                        kq = tpool.tile([128, 2 * S], F32R, tag="kqraw")
                        nc.sync.dma_start(
                            kq[:].rearrange("p (j t) -> p j t", j=2),
                            qkT_d[rq:rq + 256, bs:bs + S]
                                .rearrange("(j p) t -> p j t", p=128))
                        vt_ = tpool.tile([128, 8 * 128], F32R, tag="vt")
                        nc.sync.dma_start(
                            vt_[:].rearrange("p (kt d) -> p kt d", kt=8),
                            v_d[bs:bs + S, vcol:vcol + 128]
                               .rearrange("(kt p) d -> p kt d", p=128))
                        q_rope = kq[:, 0:S]
                        k_rope = kq[:, S:2 * S]
                        if mask_state[0] is None:
                            load_mask()
                        mask_halves = mask_state[0]
                        if b == 0 and l >= 1:
                            lc = l - 1
                            nc.sync.dma_start(wo_a[:, lc * H:(lc + 1) * H],
                                              wo[lc * 128:(lc + 1) * 128, :])
                            if l == 3:
                                nc.sync.dma_start(wo_a[:, 3 * H:4 * H],
                                                  wo[3 * 128:4 * 128, :])

                        at = apool.tile([128, S], F32R, tag=f"attn{l}")
                        for nt in range(2):
                            nq = nt * 512
                            psd = ps_d.tile([1, 512], F32, tag="dbc")
                            psav = ps_av.tile([128, 512], F32, tag="av")
                            ef_tiles = []
                            for mt in range(8):
                                pss = ps_s.tile([128, 512], F32, tag="s")
                                nc.tensor.matmul(
                                    pss[:], k_rope[:, mt * 128:(mt + 1) * 128],
                                    q_rope[:, nq:nq + 512], start=True, stop=True)
                                ef0 = epool.tile([128, 512], F32, tag="ef0")
                                nc.scalar.activation(ef0[:], pss[:], AF.Exp,
                                                     scale=SCALE)
                                ef = efpool.tile([128, 512], F32R, tag="ef")
                                mh = mask_halves[mt // 4]
                                msl = mh[:, (mt % 4) * S + nq:(mt % 4) * S + nq + 512]
                                eng = nc.gpsimd if mt == 7 else nc.vector
                                eng.tensor_tensor(ef[:], ef0[:], msl, op=ALU.mult)
                                ef_tiles.append(ef)
                                nc.tensor.matmul(
                                    psav[:], vt_[:, mt * 128:(mt + 1) * 128], ef[:],
                                    start=(mt == 0), stop=(mt == 7))
                            for mt in range(8):
                                nc.tensor.matmul(psd[:], o128[:], ef_tiles[mt][:],
                                                 start=(mt == 0), stop=(mt == 7))
                            rd = epool.tile([1, 512], F32R, tag="rd")
                            nc.vector.reciprocal(rd[:], psd[:])
                            psbc = ps_d.tile([128, 512], F32, tag="dbc")
                            nc.tensor.matmul(psbc[:], o1[:], rd[:], start=True, stop=True)
                            bcs = epool.tile([128, 512], F32, tag="bcs")
                            nc.vector.tensor_copy(bcs[:], psbc[:])
                            nc.vector.tensor_tensor(at[:, nq:nq + 512], psav[:],
                                                    bcs[:], op=ALU.mult)
                        attn_t.append(at)

                    # W_o partial for batch b's tokens (half-row output tiles)
                    for m in range(8):
                        for half in range(4):
                            os_ = opool.tile([128, 1024], F32, tag="os")
                            for n in range(2):
                                nf = half * 1024 + n * 512
                                if b == B - 1 and (2 * half + n) % 2 == 1:
                                    pso = ps_av.tile([128, 512], F32, tag="av")
                                else:
                                    pso = ps_o.tile([128, 512], F32, tag="o")
                                for l in range(HC):
                                    nc.tensor.matmul(
                                        pso[:], attn_t[l][:, m * 128:(m + 1) * 128],
                                        wo_a[:, l * H + nf:l * H + nf + 512],
                                        start=(l == 0), stop=(l == HC - 1))
                                if n % 2 == 0:
                                    nc.vector.tensor_copy(
                                        os_[:, n * 512:(n + 1) * 512], pso[:])
                                else:
                                    nc.scalar.copy(
                                        os_[:, n * 512:(n + 1) * 512], pso[:])
                            nc.sync.dma_start(
                                out_p[bs + m * 128:bs + (m + 1) * 128,
                                      half * 1024:(half + 1) * 1024], os_[:])
    nc.compile()
    return nc


def _host_prep(hidden_states, W_pack, W_o, attention_mask, position_ids):
    import ml_dtypes
    hidden_states = np.asarray(hidden_states, dtype=np.float32)
    W_pack = np.asarray(W_pack, dtype=np.float32)
    W_o = np.asarray(W_o, dtype=np.float32)
    attention_mask = np.asarray(attention_mask, dtype=np.float32)
    pos = np.asarray(position_ids)

    xT = np.ascontiguousarray(hidden_states.reshape(T, H).T)
    # exp(mask): softmax mask applied multiplicatively after exp
    maskT = np.ascontiguousarray(
        np.exp(attention_mask[:, 0].transpose(0, 2, 1)).astype(ml_dtypes.bfloat16))

    inv = (1.0 / (ROPE_BASE ** (np.arange(0, D, 2, dtype=np.float64) / D)))
    inv = np.concatenate([inv, inv])                       # [D]
    ang = pos.astype(np.float64).reshape(T)[None, :] * inv[:, None]   # [D, T]
    cosT = np.cos(ang).astype(np.float32)
    sinT = np.sin(ang).astype(np.float32)
    sinS = sinT.copy()
    sinS[:64] = -sinT[:64]
    cosT = np.ascontiguousarray(cosT)
    sinS = np.ascontiguousarray(sinS)

    in_maps = []
    for c in range(N_CORES):
        h0 = c * HC
        # wqk column order per pass p: [q_{2p}, k_{2p}, q_{2p+1}, k_{2p+1}]
        qcols = [W_pack[:, (h0 + l) * D:(h0 + l + 1) * D] for l in range(HC)]
        kcols = [W_pack[:, H + (h0 + l) * D:H + (h0 + l + 1) * D] for l in range(HC)]
        vcols = [W_pack[:, 2 * H + (h0 + l) * D:2 * H + (h0 + l + 1) * D]
                 for l in range(HC)]
        wqk_np = np.ascontiguousarray(np.concatenate(
            [qcols[0], kcols[0], qcols[1], kcols[1],
             qcols[2], kcols[2], qcols[3], kcols[3]], axis=1))
        wv_np = np.ascontiguousarray(np.concatenate(vcols, axis=1))
        wo_np = np.ascontiguousarray(W_o[h0 * D:(h0 + HC) * D, :])
        in_maps.append({
            "xT": xT, "wqk": wqk_np, "wv": wv_np, "wo": wo_np,
            "cosT": cosT, "sinS": sinS, "maskT": maskT,
        })
    return in_maps


def kernel(hidden_states, W_pack, W_o, attention_mask, position_ids):
    if "nc" not in _CACHE:
        _CACHE["nc"] = _build_module()
    nc = _CACHE["nc"]
    in_maps = _host_prep(hidden_states, W_pack, W_o, attention_mask, position_ids)
    res = bass_utils.run_bass_kernel_spmd(nc, in_maps, core_ids=list(range(N_CORES)))
    out = res.results[0]["out_p"].astype(np.float64)
    for c in range(1, N_CORES):
        out += res.results[c]["out_p"]
    return out.reshape(B, S, H).astype(np.float32)



# revision 5
# speedup vs baseline: 1.1998x; 1.1998x over previous
"""Trainium2 Bass kernel for nn_Attention_60567628808865.

Dense transformer attention block (B=4, S=1024, H=4096, NH=32, D=128):
  qkv = x @ W_pack; RoPE(q, k); causal-masked softmax attention; out @ W_o.

Sharding: tensor-parallel over heads across 8 NeuronCores. Each core computes
4 heads end-to-end; the host sums the 8 partial W_o outputs (row-sharded W_o).

Precision/performance scheme (validated on host to rel_err ~2.7e-3 vs the
2e-2 gate):
  - QKV and W_o projections run in fp8(e4m3) with the DoubleRow perf mode
    (K=256 per instruction, 0.5 cycles/row) using an exact-style two-term
    decomposition: x@W ~= x_hi@W_hi + [x_hi@W_lo + x_lo@W_hi], where
    t_hi = fp8(t*s), t_lo = fp8(t*s - t_hi). Both terms accumulate into ONE
    PSUM chain (identical scale), so the epilogue is unchanged. 48 DoubleRow
    instructions replace 32 f32r instructions per [128col x 256tok] unit:
    0.75x PE cycles.
  - hi/lo operands are slot-interleaved in a single packed tensor
    ([part, chunk, 2, free]) so the correction chain reads (hi,lo) slot pairs
    and the main chain reads (hi,hi) chunk pairs from the same SBUF bytes.
  - Attention is causal-aware: score/PV/denominator work is emitted only for
    the 20/32 key-tile x query-block units on or below the diagonal; the two
    diagonal tiles per query block are masked multiplicatively with a
    host-built exp(mask) pattern (asserted causal). q/k/v round-trip DRAM in
    bf16; scores/PV matmuls run in bf16 (same PE rate as f32r, half the DMA).
  - Softmax is unnormalized; denominators come from a ones-vector matmul
    accumulated in PSUM, broadcast back via a K=1 matmul; attention output is
    quantized to fp8 hi/lo pairs on the fly for the W_o DoubleRow chain.
  - Output partials are stored bf16; the host sum applies the global descale.
"""
import numpy as np

import concourse.bass as bass  # noqa: F401
import concourse.tile as tile
from contextlib import ExitStack
from concourse import bacc, mybir
from concourse import bass_utils

F32 = mybir.dt.float32
F32R = mybir.dt.float32r
BF16 = mybir.dt.bfloat16
F8 = mybir.dt.float8e4
AF = mybir.ActivationFunctionType
ALU = mybir.AluOpType
DR = mybir.MatmulPerfMode.DoubleRow

B, S, H, NH = 4, 1024, 4096, 32
D = H // NH          # 128
T = B * S            # 4096 tokens
N_CORES = 8
HC = NH // N_CORES   # 4 heads per core
SCALE = float(1.0 / np.sqrt(D))
ROPE_BASE = 10000.0

TB = 256             # phase-1 token block
NTB = T // TB        # 16
KC = H // 128        # 32 fp8 k-chunks of 128 features
S_X = 32.0           # x quant scale
S_W = 2048.0         # W_pack / W_o quant scale
S_A = 32.0           # attention-output quant scale
DESCALE = 1.0 / (S_X * S_W)

_CACHE = {}


def _build_module(phases=("p1", "p2")):
    nc = bacc.Bacc("TRN2", target_bir_lowering=False, debug=False,
                   num_devices=N_CORES)

    # packed fp8 inputs (see _host_prep for layouts)
    xq = nc.dram_tensor("xq", [128, NTB * KC * 2 * TB], F8, kind="ExternalInput").ap()
    wqk = nc.dram_tensor("wqk", [128, 8 * KC * 2 * 128], F8, kind="ExternalInput").ap()
    wv = nc.dram_tensor("wv", [128, 2 * KC * 2 * 256], F8, kind="ExternalInput").ap()
    wo = nc.dram_tensor("wo", [128, HC * 2 * H], F8, kind="ExternalInput").ap()
    cosT = nc.dram_tensor("cosT", [128, T], F32, kind="ExternalInput").ap()
    sinS = nc.dram_tensor("sinS", [128, T], F32, kind="ExternalInput").ap()
    maskD = nc.dram_tensor("maskD", [128, 512], BF16, kind="ExternalInput").ap()
    out_p = nc.dram_tensor("out_p", [T, H], BF16, kind="ExternalOutput").ap()

    import ml_dtypes
    ones128 = nc.inline_tensor(
        np.ones((128, 1), ml_dtypes.bfloat16), "ones128").ap()
    onesS = nc.inline_tensor(
        np.full((1, 128), S_A, np.float32), "onesS").ap().bitcast(F32R)

    with tile.TileContext(nc) as tc, \
         nc.allow_low_precision(reason="fp8/bf16 matmuls; verified vs reference"):
        with ExitStack() as octx:
            dram = octx.enter_context(tc.tile_pool(name="dram", bufs=1, space="DRAM"))
            cpool = octx.enter_context(tc.tile_pool(name="consts", bufs=1))
            # DRAM scratch: qkT rows ordered [q0,k0,q1,k1,q2,k2,q3,k3] x d
            qkT_d = dram.tile([8 * 128, T], BF16)
            v_d = dram.tile([T, HC * 128], BF16)

            o128 = cpool.tile([128, 1], BF16)
            nc.sync.dma_start(o128[:], ones128[:])
            oS = cpool.tile([1, 128], F32R)
            nc.sync.dma_start(oS[:], onesS[:])
            mask_t = cpool.tile([128, 512], BF16)
            nc.sync.dma_start(mask_t[:], maskD[:])

            # ---------------- Phase 1: QKV projection (fp8 DoubleRow) -------
            if "p1" in phases:
              with ExitStack() as ctx:
                wpool = ctx.enter_context(tc.tile_pool(name="p1w", bufs=1))
                xpool = ctx.enter_context(tc.tile_pool(name="p1x", bufs=2))
                opool = ctx.enter_context(tc.tile_pool(name="p1o", bufs=2))
                cspool = ctx.enter_context(tc.tile_pool(name="p1cs", bufs=2))
                rpool = ctx.enter_context(tc.tile_pool(name="p1rope", bufs=3))
                pqk = ctx.enter_context(tc.tile_pool(name="p1pqk", bufs=4, space="PSUM"))
                pv = ctx.enter_context(tc.tile_pool(name="p1pv", bufs=2, space="PSUM"))

                # resident weights: wqk [128, ct(8), kk(32), j(2), c(128)],
                # wv [128, ct(2), kk(32), j(2), c(256)]; j=0 -> W_lo, j=1 -> W_hi
                wqk_a = wpool.tile([128, 8, KC, 2, 128], F8, tag="wqk")
                for ct in range(8):
                    nc.sync.dma_start(
                        wqk_a[:, ct],
                        wqk[:, ct * 8192:(ct + 1) * 8192]
                            .rearrange("p (kk j c) -> p kk j c", kk=KC, j=2))
                wv_a = wpool.tile([128, 2, KC, 2, 256], F8, tag="wv")
                for ct in range(2):
                    nc.sync.dma_start(
                        wv_a[:, ct],
                        wv[:, ct * 16384:(ct + 1) * 16384]
                            .rearrange("p (kk j c) -> p kk j c", kk=KC, j=2))

                for tb in range(NTB):
                    t0 = tb * TB
                    # x pack [128, kk(32), j(2), t(256)]; j=0 -> x_hi, j=1 -> x_lo
                    xall = xpool.tile([128, KC, 2, TB], F8, tag="x")
                    nc.sync.dma_start(
                        xall[:],
                        xq[:, tb * 16384:(tb + 1) * 16384]
                            .rearrange("p (kk j t) -> p kk j t", kk=KC, j=2))
                    cos_tb = cspool.tile([128, TB], F32, tag="cos")
                    nc.sync.dma_start(cos_tb[:], cosT[:, t0:t0 + TB])
                    sin_tb = cspool.tile([128, TB], F32, tag="sin")
                    nc.sync.dma_start(sin_tb[:], sinS[:, t0:t0 + TB])

                    qs_all = opool.tile([128, 8, TB], BF16, tag="qs")
                    for i in range(8):
                        ps = pqk.tile([128, TB], F32, tag="qk")
                        for c in range(16):
                            nc.tensor.matmul(
                                ps[:], wqk_a[:, i, 2 * c:2 * c + 2, 1, :],
                                xall[:, 2 * c:2 * c + 2, 0, :],
                                start=(c == 0), stop=False, perf_mode=DR)
                        for kk in range(KC):
                            nc.tensor.matmul(
                                ps[:], wqk_a[:, i, kk, :, :],
                                xall[:, kk, :, :],
                                start=False, stop=(kk == KC - 1), perf_mode=DR)
                        # RoPE epilogue (psum scale folded into cos/sin tables)
                        rot = rpool.tile([128, TB], F32, tag="rot")
                        nc.scalar.copy(rot[0:64, :], ps[64:128, :])
                        nc.vector.tensor_copy(rot[64:128, :], ps[0:64, :])
                        m1_ = rpool.tile([128, TB], F32, tag="m1")
                        nc.vector.tensor_tensor(m1_[:], ps[:], cos_tb[:], op=ALU.mult)
                        m2_ = rpool.tile([128, TB], F32, tag="m2")
                        nc.vector.tensor_tensor(m2_[:], rot[:], sin_tb[:], op=ALU.mult)
                        nc.vector.tensor_tensor(qs_all[:, i, :], m1_[:], m2_[:],
                                                op=ALU.add)
                    nc.sync.dma_start(
                        qkT_d[:, t0:t0 + TB].rearrange("(i p) t -> p i t", p=128),
                        qs_all[:])

                    vs_all = opool.tile([128, 2, 2, 256], BF16, tag="vs")
                    for th in range(2):
                        for ch in range(2):
                            ps = pv.tile([128, 256], F32, tag="v")
                            for c in range(16):
                                nc.tensor.matmul(
                                    ps[:],
                                    xall[:, 2 * c:2 * c + 2, 0,
                                         th * 128:(th + 1) * 128],
                                    wv_a[:, ch, 2 * c:2 * c + 2, 1, :],
                                    start=(c == 0), stop=False, perf_mode=DR)
                            for kk in range(KC):
                                nc.tensor.matmul(
                                    ps[:],
                                    xall[:, kk, :, th * 128:(th + 1) * 128],
                                    wv_a[:, ch, kk, :, :],
                                    start=False, stop=(kk == KC - 1), perf_mode=DR)
                            nc.scalar.activation(vs_all[:, th, ch, :], ps[:],
                                                 AF.Copy, scale=DESCALE)
                    nc.sync.dma_start(
                        v_d[t0:t0 + TB, :]
                            .rearrange("(th p) (ch c) -> p th ch c", p=128, ch=2),
                        vs_all[:])

            # ---------------- Phase 2: attention + W_o ----------------------
            if "p2" in phases:
              with ExitStack() as ctx:
                wopool = ctx.enter_context(tc.tile_pool(name="p2wo", bufs=1))
                apool = ctx.enter_context(tc.tile_pool(name="p2a", bufs=2))
                kqpool = ctx.enter_context(tc.tile_pool(name="p2kq", bufs=2))
                vtpool = ctx.enter_context(tc.tile_pool(name="p2vt", bufs=2))
                efpool = ctx.enter_context(tc.tile_pool(name="p2ef", bufs=10))
                tpool = ctx.enter_context(tc.tile_pool(name="p2t", bufs=3))
                rpool2 = ctx.enter_context(tc.tile_pool(name="p2rd", bufs=3))
                opool = ctx.enter_context(tc.tile_pool(name="p2o", bufs=2))
                ps_s = ctx.enter_context(tc.tile_pool(name="p2ps", bufs=3, space="PSUM"))
                ps_av = ctx.enter_context(tc.tile_pool(name="p2pav", bufs=2, space="PSUM"))
                ps_d = ctx.enter_context(tc.tile_pool(name="p2pd", bufs=1, space="PSUM"))
                ps_o = ctx.enter_context(tc.tile_pool(name="p2po", bufs=2, space="PSUM"))

                # W_o resident: [128, h(4), j(2), c(4096)]; j=0 -> hi, j=1 -> lo
                wo_a = wopool.tile([128, HC, 2, H], F8, tag="wo")
                for h in range(HC):
                    nc.sync.dma_start(
                        wo_a[:, h],
                        wo[:, h * 2 * H:(h + 1) * 2 * H]
                            .rearrange("p (j c) -> p j c", j=2))

                for b in range(B):
                    bs = b * S
                    # attn pack [128, lh(2), l(4), t(1024)]; lh=0 -> lo, 1 -> hi
                    apack = apool.tile([128, 2, HC, S], F8, tag="apack")
                    for l in range(HC):
                        kq = kqpool.tile([128, 2, S], BF16, tag="kq")
                        nc.sync.dma_start(
                            kq[:],
                            qkT_d[l * 256:(l + 1) * 256, bs:bs + S]
                                .rearrange("(j p) t -> p j t", p=128))
                        vt = vtpool.tile([128, 8, 128], BF16, tag="vt")
                        nc.sync.dma_start(
                            vt[:],
                            v_d[bs:bs + S, l * 128:(l + 1) * 128]
                                .rearrange("(kt p) d -> p kt d", p=128))

                        pending = [None]
                        for qb in range(4):
                            u = 2 * qb + 2
                            q_sl = kq[:, 0, qb * 256:(qb + 1) * 256]
                            efs = []  # per key-tile [128,256] bf16 slices
                            for g in range(u // 2):
                                pss = ps_s.tile([128, 512], F32, tag="s")
                                for sHalf in range(2):
                                    mt = 2 * g + sHalf
                                    nc.tensor.matmul(
                                        pss[:, sHalf * 256:(sHalf + 1) * 256],
                                        kq[:, 1, mt * 128:(mt + 1) * 128],
                                        q_sl, start=True, stop=True)
                                ef = efpool.tile([128, 512], BF16, tag="ef")
                                nc.scalar.activation(ef[:], pss[:], AF.Exp,
                                                     scale=SCALE)
                                if g == qb:  # diagonal pair: multiplicative mask
                                    efm = efpool.tile([128, 512], BF16, tag="efm")
                                    nc.gpsimd.tensor_tensor(efm[:], ef[:],
                                                            mask_t[:], op=ALU.mult)
                                    ef = efm
                                efs.append(ef[:, 0:256])
                                efs.append(ef[:, 256:512])
                            if pending[0] is not None:
                                pending[0]()
                                pending[0] = None
                            psav = ps_av.tile([128, 256], F32, tag="av")
                            for mt in range(u):
                                nc.tensor.matmul(
                                    psav[:], vt[:, mt, :], efs[mt],
                                    start=(mt == 0), stop=(mt == u - 1))
                            psd = ps_d.tile([1, 256], F32, tag="dbc")
                            for mt in range(u):
                                nc.tensor.matmul(
                                    psd[:], o128[:], efs[mt],
                                    start=(mt == 0), stop=(mt == u - 1))
                            rd = rpool2.tile([1, 256], F32R, tag="rd")
                            nc.vector.reciprocal(rd[:], psd[:])

                            def make_epilogue(qb=qb, psav=psav, rd=rd, l=l,
                                              apack=apack):
                                def emit():
                                    psbc = ps_o.tile([128, 256], F32, tag="o")
                                    nc.tensor.matmul(psbc[:], oS[:], rd[:],
                                                     start=True, stop=True)
                                    t_ = tpool.tile([128, 256], F32, tag="t")
                                    nc.vector.tensor_tensor(t_[:], psav[:],
                                                            psbc[:], op=ALU.mult)
                                    q0 = qb * 256
                                    hi = apack[:, 1, l, q0:q0 + 256]
                                    nc.vector.tensor_copy(hi, t_[:])
                                    nc.vector.tensor_tensor(
                                        apack[:, 0, l, q0:q0 + 256], t_[:], hi,
                                        op=ALU.subtract)
                                return emit
                            pending[0] = make_epilogue()
                        pending[0]()

                    # W_o projection for batch b (fp8 DoubleRow main+corr)
                    for m in range(8):
                        osb = opool.tile([128, 16, 256], BF16, tag="osb")
                        msl = slice(m * 128, (m + 1) * 128)
                        for ncol in range(16):
                            pso = ps_o.tile([128, 256], F32, tag="o")
                            csl = slice(ncol * 256, (ncol + 1) * 256)
                            for c in range(2):
                                nc.tensor.matmul(
                                    pso[:], apack[:, 1, 2 * c:2 * c + 2, msl],
                                    wo_a[:, 2 * c:2 * c + 2, 0, csl],
                                    start=(c == 0), stop=False, perf_mode=DR)
                            for h in range(HC):
                                nc.tensor.matmul(
                                    pso[:], apack[:, :, h, msl],
                                    wo_a[:, h, :, csl],
                                    start=False, stop=(h == HC - 1), perf_mode=DR)
                            if ncol % 2 == 0:
                                nc.vector.tensor_copy(osb[:, ncol, :], pso[:])
                            else:
                                nc.scalar.copy(osb[:, ncol, :], pso[:])
                        nc.sync.dma_start(
                            out_p[bs + m * 128:bs + (m + 1) * 128, :],
                            osb[:].rearrange("p nc c -> p (nc c)"))
    nc.compile()
    return nc


def _q8hl(a, scale):
    """Quantize to fp8 e4m3 hi/lo pair at a shared scale."""
    import ml_dtypes
    hi = (a * scale).astype(ml_dtypes.float8_e4m3)
    lo = ((a * scale) - hi.astype(np.float32)).astype(ml_dtypes.float8_e4m3)
    return hi, lo


def _host_prep(hidden_states, W_pack, W_o, attention_mask, position_ids):
    import ml_dtypes
    x = np.asarray(hidden_states, dtype=np.float32).reshape(T, H)
    W_pack = np.asarray(W_pack, dtype=np.float32)
    W_o = np.asarray(W_o, dtype=np.float32)
    mask = np.asarray(attention_mask, dtype=np.float32)
    pos = np.asarray(position_ids)

    # causal structure is hardcoded in the kernel; verify it holds
    m0 = mask[0, 0]
    iu = np.triu_indices(S, 1)
    assert (m0[iu] < -1e8).all() and (np.tril(m0) == 0).all(), \
        "kernel requires the standard causal mask"

    # x pack: [128p, tb, kk, j(hi,lo), t] -> flat [128, NTB*KC*2*TB]
    xh, xl = _q8hl(x, S_X)
    xv_h = xh.reshape(NTB, TB, KC, 128).transpose(3, 0, 2, 1)
    xv_l = xl.reshape(NTB, TB, KC, 128).transpose(3, 0, 2, 1)
    xq_np = np.empty((128, NTB, KC, 2, TB), ml_dtypes.float8_e4m3)
    xq_np[:, :, :, 0, :] = xv_h
    xq_np[:, :, :, 1, :] = xv_l
    xq_np = np.ascontiguousarray(xq_np.reshape(128, -1))

    # rope tables with the fp8 descale folded in; rotate-half sign in sinS
    inv = 1.0 / (ROPE_BASE ** (np.arange(0, D, 2, dtype=np.float64) / D))
    inv = np.concatenate([inv, inv])
    ang = pos.astype(np.float64).reshape(T)[None, :] * inv[:, None]   # [D, T]
    cosT_np = np.ascontiguousarray((np.cos(ang) * DESCALE).astype(np.float32))
    sinT = (np.sin(ang) * DESCALE).astype(np.float32)
    sinS_np = sinT.copy()
    sinS_np[:64] = -sinT[:64]
    sinS_np = np.ascontiguousarray(sinS_np)

    # diagonal exp-mask patterns [128p(key), s(2)*256(query)] bf16
    em = np.exp(m0)
    maskD_np = np.empty((128, 2, 256), ml_dtypes.bfloat16)
    maskD_np[:, 0, :] = em[0:256, 0:128].T       # offset 0 pattern
    maskD_np[:, 1, :] = em[0:256, 128:256].T     # offset 128 pattern
    maskD_np = np.ascontiguousarray(maskD_np.reshape(128, 512))

    in_maps = []
    for core in range(N_CORES):
        h0 = core * HC
        # wqk cols ordered [q0,k0,q1,k1,q2,k2,q3,k3] per head slice
        cols = []
        for l in range(HC):
            cols.append(W_pack[:, (h0 + l) * D:(h0 + l + 1) * D])
            cols.append(W_pack[:, H + (h0 + l) * D:H + (h0 + l + 1) * D])
        wqk_f = np.concatenate(cols, axis=1)              # [H, 1024]
        wh, wl = _q8hl(wqk_f, S_W)
        wv_h = wh.reshape(KC, 128, 8, 128).transpose(1, 2, 0, 3)
        wv_l = wl.reshape(KC, 128, 8, 128).transpose(1, 2, 0, 3)
        wqk_np = np.empty((128, 8, KC, 2, 128), ml_dtypes.float8_e4m3)
        wqk_np[:, :, :, 0, :] = wv_l
        wqk_np[:, :, :, 1, :] = wv_h
        wqk_np = np.ascontiguousarray(wqk_np.reshape(128, -1))

        wv_f = np.concatenate(
            [W_pack[:, 2 * H + (h0 + l) * D:2 * H + (h0 + l + 1) * D]
             for l in range(HC)], axis=1)                 # [H, 512]
        wh, wl = _q8hl(wv_f, S_W)
        wvv_h = wh.reshape(KC, 128, 2, 256).transpose(1, 2, 0, 3)
        wvv_l = wl.reshape(KC, 128, 2, 256).transpose(1, 2, 0, 3)
        wv_np = np.empty((128, 2, KC, 2, 256), ml_dtypes.float8_e4m3)
        wv_np[:, :, :, 0, :] = wvv_l
        wv_np[:, :, :, 1, :] = wvv_h
        wv_np = np.ascontiguousarray(wv_np.reshape(128, -1))

        wo_f = W_o[h0 * D:(h0 + HC) * D, :]               # [512, H]
        wh, wl = _q8hl(wo_f, S_W)
        wov_h = wh.reshape(HC, 128, H).transpose(1, 0, 2)
        wov_l = wl.reshape(HC, 128, H).transpose(1, 0, 2)
        wo_np = np.empty((128, HC, 2, H), ml_dtypes.float8_e4m3)
        wo_np[:, :, 0, :] = wov_h
        wo_np[:, :, 1, :] = wov_l
        wo_np = np.ascontiguousarray(wo_np.reshape(128, -1))

        in_maps.append({
            "xq": xq_np, "wqk": wqk_np, "wv": wv_np, "wo": wo_np,
            "cosT": cosT_np, "sinS": sinS_np, "maskD": maskD_np,
        })
    return in_maps


def kernel(hidden_states, W_pack, W_o, attention_mask, position_ids):
    if "nc" not in _CACHE:
        _CACHE["nc"] = _build_module()
    nc = _CACHE["nc"]
    in_maps = _host_prep(hidden_states, W_pack, W_o, attention_mask, position_ids)
    res = bass_utils.run_bass_kernel_spmd(nc, in_maps, core_ids=list(range(N_CORES)))
    out = res.results[0]["out_p"].astype(np.float32)
    for c in range(1, N_CORES):
        out += res.results[c]["out_p"]
    out *= 1.0 / (S_A * S_W)
    return out.reshape(B, S, H).astype(np.float32)


# revision 14
# speedup vs baseline: 1.3663x; 1.1388x over previous
"""Trainium2 Bass kernel for nn_Attention_60567628808865.

Dense transformer attention block (B=4, S=1024, H=4096, NH=32, D=128):
  qkv = x @ W_pack; RoPE(q, k); causal-masked softmax attention; out @ W_o.

Sharding: tensor-parallel over heads across 8 NeuronCores. Each core computes
4 heads end-to-end; the host sums the 8 partial W_o outputs (row-sharded W_o).

Precision/performance scheme (validated on host to rel_err ~2.7e-3 vs the
2e-2 gate):
  - QKV and W_o projections run in fp8(e4m3) with the DoubleRow perf mode
    (K=256 per instruction, 0.5 cycles/row) using an exact-style two-term
    decomposition: x@W ~= x_hi@W_hi + [x_hi@W_lo + x_lo@W_hi], where
    t_hi = fp8(t*s), t_lo = fp8(t*s - t_hi). Both terms accumulate into ONE
    PSUM chain (identical scale), so the epilogue is unchanged. 48 DoubleRow
    instructions replace 32 f32r instructions per [128col x 256tok] unit:
    0.75x PE cycles.
  - hi/lo operands are slot-interleaved in a single packed tensor
    ([part, chunk, 2, free]) so the correction chain reads (hi,lo) slot pairs
    and the main chain reads (hi,hi) chunk pairs from the same SBUF bytes.
  - Attention is causal-aware: score/PV/denominator work is emitted only for
    the 20/32 key-tile x query-block units on or below the diagonal; the two
    diagonal tiles per query block are masked multiplicatively with a
    host-built exp(mask) pattern (asserted causal). q/k/v round-trip DRAM in
    bf16; scores/PV matmuls run in bf16 (same PE rate as f32r, half the DMA).
  - Softmax is unnormalized; denominators come from a ones-vector matmul
    accumulated in PSUM, broadcast back via a K=1 matmul; attention output is
    quantized to fp8 hi/lo pairs on the fly for the W_o DoubleRow chain.
  - Output partials are stored bf16; the host sum applies the global descale.
"""
import numpy as np

import concourse.bass as bass  # noqa: F401
import concourse.tile as tile
from contextlib import ExitStack
from concourse import bacc, mybir
from concourse import bass_utils

F32 = mybir.dt.float32
F32R = mybir.dt.float32r
BF16 = mybir.dt.bfloat16
F8 = mybir.dt.float8e4
AF = mybir.ActivationFunctionType
ALU = mybir.AluOpType
DR = mybir.MatmulPerfMode.DoubleRow

B, S, H, NH = 4, 1024, 4096, 32
D = H // NH          # 128
T = B * S            # 4096 tokens
N_CORES = 8
HC = NH // N_CORES   # 4 heads per core
SCALE = float(1.0 / np.sqrt(D))
ROPE_BASE = 10000.0

TB = 256             # phase-1 token block
NTB = T // TB        # 16
KC = H // 128        # 32 fp8 k-chunks of 128 features
S_X = 32.0           # x quant scale
S_W = 2048.0         # W_pack / W_o quant scale
S_A = 32.0           # attention-output quant scale
DESCALE = 1.0 / (S_X * S_W)

_CACHE = {}


def _build_module(phases=("p1", "p2")):
    nc = bacc.Bacc("TRN2", target_bir_lowering=False, debug=False,
                   num_devices=N_CORES)

    # packed fp8 inputs (see _host_prep for layouts)
    xq = nc.dram_tensor("xq", [128, NTB * KC * 2 * TB], F8, kind="ExternalInput").ap()
    wqk = nc.dram_tensor("wqk", [128, 8 * KC * 2 * 128], F8, kind="ExternalInput").ap()
    wv = nc.dram_tensor("wv", [128, 2 * KC * 2 * 256], F8, kind="ExternalInput").ap()
    wo = nc.dram_tensor("wo", [128, HC * 2 * H], F8, kind="ExternalInput").ap()
    cosT = nc.dram_tensor("cosT", [128, T], F32, kind="ExternalInput").ap()
    sinS = nc.dram_tensor("sinS", [128, T], F32, kind="ExternalInput").ap()
    maskD = nc.dram_tensor("maskD", [128, 512], BF16, kind="ExternalInput").ap()
    out_p = nc.dram_tensor("out_p", [T, H], BF16, kind="ExternalOutput").ap()

    import ml_dtypes
    ones128 = nc.inline_tensor(
        np.ones((128, 1), ml_dtypes.bfloat16), "ones128").ap()
    onesS = nc.inline_tensor(
        np.full((1, 128), S_A, np.float32), "onesS").ap().bitcast(F32R)

    with tile.TileContext(nc) as tc, \
         nc.allow_low_precision(reason="fp8/bf16 matmuls; verified vs reference"):
        with ExitStack() as octx:
            dram = octx.enter_context(tc.tile_pool(name="dram", bufs=1, space="DRAM"))
            cpool = octx.enter_context(tc.tile_pool(name="consts", bufs=1))
            # DRAM scratch: qkT rows ordered [q0,k0,q1,k1,q2,k2,q3,k3] x d
            qkT_d = dram.tile([8 * 128, T], BF16)
            v_d = dram.tile([T, HC * 128], BF16)

            o128 = cpool.tile([128, 1], BF16)
            nc.sync.dma_start(o128[:], ones128[:])
            oS = cpool.tile([1, 128], F32R)
            nc.sync.dma_start(oS[:], onesS[:])
            mask_t = cpool.tile([128, 512], BF16)
            nc.sync.dma_start(mask_t[:], maskD[:])

            # phase-2 tiles prefetched during phase 1 (wo_a has no deps; the
            # first head's kq/vt depend on the tb0-3 scratch stores)
            wopool = octx.enter_context(tc.tile_pool(name="p2wo", bufs=1))
            kqpool = octx.enter_context(tc.tile_pool(name="p2kq", bufs=2))
            vtpool = octx.enter_context(tc.tile_pool(name="p2vt", bufs=2))
            _wo_a = [None]
            _first_kv = [None]

            def load_kv(b, l):
                bs = b * S
                kq = kqpool.tile([128, 2, S], BF16, tag="kq")
                nc.sync.dma_start(
                    kq[:],
                    qkT_d[l * 256:(l + 1) * 256, bs:bs + S]
                        .rearrange("(j p) t -> p j t", p=128))
                vt = vtpool.tile([128, 8, 128], BF16, tag="vt")
                nc.sync.dma_start(
                    vt[:],
                    v_d[bs:bs + S, l * 128:(l + 1) * 128]
                        .rearrange("(kt p) d -> p kt d", p=128))
                return kq, vt

            def prefetch_wo():
                # W_o resident: [128, h(4), j(2), c(4096)]; j=0 -> hi, 1 -> lo
                wo_a = wopool.tile([128, HC, 2, H], F8, tag="wo")
                for h in range(HC):
                    nc.sync.dma_start(
                        wo_a[:, h],
                        wo[:, h * 2 * H:(h + 1) * 2 * H]
                            .rearrange("p (j c) -> p j c", j=2))
                _wo_a[0] = wo_a

            def prefetch_kv():
                _first_kv[0] = load_kv(0, 0)

            # ---------------- Phase 1: QKV projection (fp8 DoubleRow) -------
            if "p1" in phases:
              with ExitStack() as ctx:
                wpool = ctx.enter_context(tc.tile_pool(name="p1w", bufs=1))
                xpool = ctx.enter_context(tc.tile_pool(name="p1x", bufs=2))
                opool = ctx.enter_context(tc.tile_pool(name="p1o", bufs=2))
                cspool = ctx.enter_context(tc.tile_pool(name="p1cs", bufs=2))
                rpool = ctx.enter_context(tc.tile_pool(name="p1rope", bufs=3))
                pqk = ctx.enter_context(tc.tile_pool(name="p1pqk", bufs=4, space="PSUM"))
                pv = ctx.enter_context(tc.tile_pool(name="p1pv", bufs=2, space="PSUM"))

                def load_tb(tb):
                    t0 = tb * TB
                    # x pack [128, kk(32), j(2), t(256)]; j=0 -> x_hi, j=1 -> x_lo
                    xall = xpool.tile([128, KC, 2, TB], F8, tag="x")
                    nc.sync.dma_start(
                        xall[:],
                        xq[:, tb * 16384:(tb + 1) * 16384]
                            .rearrange("p (kk j t) -> p kk j t", kk=KC, j=2))
                    cos_tb = cspool.tile([128, TB], F32, tag="cos")
                    nc.sync.dma_start(cos_tb[:], cosT[:, t0:t0 + TB])
                    sin_tb = cspool.tile([128, TB], F32, tag="sin")
                    nc.sync.dma_start(sin_tb[:], sinS[:, t0:t0 + TB])
                    return xall, cos_tb, sin_tb

                # tb0 inputs first (first chain needs x + wqk ct0 only), then
                # resident weights: wqk [128, ct(8), kk(32), j(2), c(128)],
                # wv [128, ct(2), kk(32), j(2), c(256)]; j=0 -> W_lo, j=1 -> W_hi
                tb0_inputs = load_tb(0)
                wqk_a = wpool.tile([128, 8, KC, 2, 128], F8, tag="wqk")
                wv_a = wpool.tile([128, 2, KC, 2, 256], F8, tag="wv")
                for ct in range(8):
                    nc.sync.dma_start(
                        wqk_a[:, ct],
                        wqk[:, ct * 8192:(ct + 1) * 8192]
                            .rearrange("p (kk j c) -> p kk j c", kk=KC, j=2))
                for cv in range(2):
                    nc.sync.dma_start(
                        wv_a[:, cv],
                        wv[:, cv * 16384:(cv + 1) * 16384]
                            .rearrange("p (kk j c) -> p kk j c", kk=KC, j=2))

                for tb in range(NTB):
                    t0 = tb * TB
                    if tb == 0:
                        xall, cos_tb, sin_tb = tb0_inputs
                    else:
                        xall, cos_tb, sin_tb = load_tb(tb)
                    if tb == 1:
                        prefetch_wo()
                    elif tb == 4:
                        prefetch_kv()

                    qs_all = opool.tile([128, 8, TB], BF16, tag="qs")
                    for i in range(8):
                        ps = pqk.tile([128, TB], F32, tag="qk")
                        for c in range(16):
                            nc.tensor.matmul(
                                ps[:], wqk_a[:, i, 2 * c:2 * c + 2, 1, :],
                                xall[:, 2 * c:2 * c + 2, 0, :],
                                start=(c == 0), stop=False, perf_mode=DR)
                        for kk in range(KC):
                            nc.tensor.matmul(
                                ps[:], wqk_a[:, i, kk, :, :],
                                xall[:, kk, :, :],
                                start=False, stop=(kk == KC - 1), perf_mode=DR)
                        # RoPE epilogue (psum scale folded into cos/sin tables)
                        rot = rpool.tile([128, TB], F32, tag="rot")
                        nc.scalar.copy(rot[0:64, :], ps[64:128, :])
                        nc.vector.tensor_copy(rot[64:128, :], ps[0:64, :])
                        m1_ = rpool.tile([128, TB], F32, tag="m1")
                        nc.vector.tensor_tensor(m1_[:], ps[:], cos_tb[:], op=ALU.mult)
                        m2_ = rpool.tile([128, TB], F32, tag="m2")
                        nc.vector.tensor_tensor(m2_[:], rot[:], sin_tb[:], op=ALU.mult)
                        nc.vector.tensor_tensor(qs_all[:, i, :], m1_[:], m2_[:],
                                                op=ALU.add)
                    nc.sync.dma_start(
                        qkT_d[:, t0:t0 + TB].rearrange("(i p) t -> p i t", p=128),
                        qs_all[:])

                    vs_all = opool.tile([128, 2, 2, 256], BF16, tag="vs")
                    for th in range(2):
                        for ch in range(2):
                            ps = pv.tile([128, 256], F32, tag="v")
                            for c in range(16):
                                nc.tensor.matmul(
                                    ps[:],
                                    xall[:, 2 * c:2 * c + 2, 0,
                                         th * 128:(th + 1) * 128],
                                    wv_a[:, ch, 2 * c:2 * c + 2, 1, :],
                                    start=(c == 0), stop=False, perf_mode=DR)
                            for kk in range(KC):
                                nc.tensor.matmul(
                                    ps[:],
                                    xall[:, kk, :, th * 128:(th + 1) * 128],
                                    wv_a[:, ch, kk, :, :],
                                    start=False, stop=(kk == KC - 1), perf_mode=DR)
                            nc.scalar.activation(vs_all[:, th, ch, :], ps[:],
                                                 AF.Copy, scale=DESCALE)
                    nc.sync.dma_start(
                        v_d[t0:t0 + TB, :]
                            .rearrange("(th p) (ch c) -> p th ch c", p=128, ch=2),
                        vs_all[:])

            # ---------------- Phase 2: attention + W_o ----------------------
            if "p2" in phases:
              with ExitStack() as ctx:
                apool = ctx.enter_context(tc.tile_pool(name="p2a", bufs=2))
                efpool = ctx.enter_context(tc.tile_pool(name="p2ef", bufs=10))
                tpool = ctx.enter_context(tc.tile_pool(name="p2t", bufs=3))
                rpool2 = ctx.enter_context(tc.tile_pool(name="p2rd", bufs=3))
                opool = ctx.enter_context(tc.tile_pool(name="p2o", bufs=2))
                ps_s = ctx.enter_context(tc.tile_pool(name="p2ps", bufs=3, space="PSUM"))
                ps_av = ctx.enter_context(tc.tile_pool(name="p2pav", bufs=2, space="PSUM"))
                ps_d = ctx.enter_context(tc.tile_pool(name="p2pd", bufs=1, space="PSUM"))
                ps_o = ctx.enter_context(tc.tile_pool(name="p2po", bufs=2, space="PSUM"))

                wo_a = _wo_a[0]
                for b in range(B):
                    bs = b * S
                    # attn pack [128, lh(2), l(4), t(1024)]; lh=0 -> lo, 1 -> hi
                    apack = apool.tile([128, 2, HC, S], F8, tag="apack")
                    for l in range(HC):
                        if b == 0 and l == 0:
                            kq, vt = _first_kv[0]
                        else:
                            kq, vt = load_kv(b, l)

                        pending = [None]
                        psd_l = ps_d.tile([1, 512], F32, tag="dbc")
                        for qb in range(4):
                            u = 2 * qb + 2
                            q_sl = kq[:, 0, qb * 256:(qb + 1) * 256]
                            efs = []  # per key-tile [128,256] bf16 slices
                            for g in range(u // 2):
                                pss = ps_s.tile([128, 512], F32, tag="s")
                                for sHalf in range(2):
                                    mt = 2 * g + sHalf
                                    nc.tensor.matmul(
                                        pss[:, sHalf * 256:(sHalf + 1) * 256],
                                        kq[:, 1, mt * 128:(mt + 1) * 128],
                                        q_sl, start=True, stop=True)
                                ef = efpool.tile([128, 512], BF16, tag="ef")
                                nc.scalar.activation(ef[:], pss[:], AF.Exp,
                                                     scale=SCALE)
                                if g == qb:  # diagonal pair: multiplicative mask
                                    efm = efpool.tile([128, 512], BF16, tag="efm")
                                    nc.gpsimd.tensor_tensor(efm[:], ef[:],
                                                            mask_t[:], op=ALU.mult)
                                    ef = efm
                                efs.append(ef[:, 0:256])
                                efs.append(ef[:, 256:512])
                            if pending[0] is not None:
                                pending[0]()
                                pending[0] = None
                            psav = ps_av.tile([128, 256], F32, tag="av")
                            for mt in range(u):
                                nc.tensor.matmul(
                                    psav[:], vt[:, mt, :], efs[mt],
                                    start=(mt == 0), stop=(mt == u - 1))
                            psd = psd_l[:, (qb % 2) * 256:(qb % 2 + 1) * 256]
                            for mt in range(u):
                                nc.tensor.matmul(
                                    psd, o128[:], efs[mt],
                                    start=(mt == 0), stop=(mt == u - 1))
                            rd = rpool2.tile([1, 256], F32R, tag="rd")
                            nc.vector.reciprocal(rd[:], psd)

                            def make_epilogue(qb=qb, psav=psav, rd=rd, l=l,
                                              apack=apack):
                                def emit():
                                    psbc = ps_o.tile([128, 256], F32, tag="o")
                                    nc.tensor.matmul(psbc[:], oS[:], rd[:],
                                                     start=True, stop=True)
                                    t_ = tpool.tile([128, 256], F32, tag="t")
                                    nc.vector.tensor_tensor(t_[:], psav[:],
                                                            psbc[:], op=ALU.mult)
                                    q0 = qb * 256
                                    hi = apack[:, 1, l, q0:q0 + 256]
                                    nc.vector.tensor_copy(hi, t_[:])
                                    nc.vector.tensor_tensor(
                                        apack[:, 0, l, q0:q0 + 256], t_[:], hi,
                                        op=ALU.subtract)
                                return emit
                            pending[0] = make_epilogue()
                        pending[0]()

                    # W_o projection for batch b (fp8 DoubleRow main+corr);
                    # two 256-col chains per PSUM bank, one wide copy each
                    for m in range(8):
                        osb = opool.tile([128, 8, 512], BF16, tag="osb")
                        msl = slice(m * 128, (m + 1) * 128)
                        for pair in range(8):
                            pso = ps_o.tile([128, 512], F32, tag="o")
                            for half in range(2):
                                csl = slice((2 * pair + half) * 256,
                                            (2 * pair + half + 1) * 256)
                                po = pso[:, half * 256:(half + 1) * 256]
                                for c in range(2):
                                    nc.tensor.matmul(
                                        po, apack[:, 1, 2 * c:2 * c + 2, msl],
                                        wo_a[:, 2 * c:2 * c + 2, 0, csl],
                                        start=(c == 0), stop=False, perf_mode=DR)
                                for h in range(HC):
                                    nc.tensor.matmul(
                                        po, apack[:, :, h, msl],
                                        wo_a[:, h, :, csl],
                                        start=False, stop=(h == HC - 1),
                                        perf_mode=DR)
                            if pair % 2 == 0:
                                nc.vector.tensor_copy(osb[:, pair, :], pso[:])
                            else:
                                nc.scalar.copy(osb[:, pair, :], pso[:])
                        nc.sync.dma_start(
                            out_p[bs + m * 128:bs + (m + 1) * 128, :],
                            osb[:].rearrange("p nc c -> p (nc c)"))
    nc.compile()
    return nc


def _q8hl(a, scale):
    """Quantize to fp8 e4m3 hi/lo pair at a shared scale."""
    import ml_dtypes
    hi = (a * scale).astype(ml_dtypes.float8_e4m3)
    lo = ((a * scale) - hi.astype(np.float32)).astype(ml_dtypes.float8_e4m3)
    return hi, lo


def _host_prep(hidden_states, W_pack, W_o, attention_mask, position_ids):
    import ml_dtypes
    x = np.asarray(hidden_states, dtype=np.float32).reshape(T, H)
    W_pack = np.asarray(W_pack, dtype=np.float32)
    W_o = np.asarray(W_o, dtype=np.float32)
    mask = np.asarray(attention_mask, dtype=np.float32)
    pos = np.asarray(position_ids)

    # causal structure is hardcoded in the kernel; verify it holds
    m0 = mask[0, 0]
    iu = np.triu_indices(S, 1)
    assert (m0[iu] < -1e8).all() and (np.tril(m0) == 0).all(), \
        "kernel requires the standard causal mask"

    # x pack: [128p, tb, kk, j(hi,lo), t] -> flat [128, NTB*KC*2*TB]
    xh, xl = _q8hl(x, S_X)
    xv_h = xh.reshape(NTB, TB, KC, 128).transpose(3, 0, 2, 1)
    xv_l = xl.reshape(NTB, TB, KC, 128).transpose(3, 0, 2, 1)
    xq_np = np.empty((128, NTB, KC, 2, TB), ml_dtypes.float8_e4m3)
    xq_np[:, :, :, 0, :] = xv_h
    xq_np[:, :, :, 1, :] = xv_l
    xq_np = np.ascontiguousarray(xq_np.reshape(128, -1))

    # rope tables with the fp8 descale folded in; rotate-half sign in sinS
    inv = 1.0 / (ROPE_BASE ** (np.arange(0, D, 2, dtype=np.float64) / D))
    inv = np.concatenate([inv, inv])
    ang = pos.astype(np.float64).reshape(T)[None, :] * inv[:, None]   # [D, T]
    cosT_np = np.ascontiguousarray((np.cos(ang) * DESCALE).astype(np.float32))
    sinT = (np.sin(ang) * DESCALE).astype(np.float32)
    sinS_np = sinT.copy()
    sinS_np[:64] = -sinT[:64]
    sinS_np = np.ascontiguousarray(sinS_np)

    # diagonal exp-mask patterns [128p(key), s(2)*256(query)] bf16
    em = np.exp(m0)
    maskD_np = np.empty((128, 2, 256), ml_dtypes.bfloat16)
    maskD_np[:, 0, :] = em[0:256, 0:128].T       # offset 0 pattern
    maskD_np[:, 1, :] = em[0:256, 128:256].T     # offset 128 pattern
    maskD_np = np.ascontiguousarray(maskD_np.reshape(128, 512))

    in_maps = []
    for core in range(N_CORES):
        h0 = core * HC
        # wqk cols ordered [q0,k0,q1,k1,q2,k2,q3,k3] per head slice
        cols = []
        for l in range(HC):
            cols.append(W_pack[:, (h0 + l) * D:(h0 + l + 1) * D])
            cols.append(W_pack[:, H + (h0 + l) * D:H + (h0 + l + 1) * D])
        wqk_f = np.concatenate(cols, axis=1)              # [H, 1024]
        wh, wl = _q8hl(wqk_f, S_W)
        wv_h = wh.reshape(KC, 128, 8, 128).transpose(1, 2, 0, 3)
        wv_l = wl.reshape(KC, 128, 8, 128).transpose(1, 2, 0, 3)
        wqk_np = np.empty((128, 8, KC, 2, 128), ml_dtypes.float8_e4m3)
        wqk_np[:, :, :, 0, :] = wv_l
        wqk_np[:, :, :, 1, :] = wv_h
        wqk_np = np.ascontiguousarray(wqk_np.reshape(128, -1))

        wv_f = np.concatenate(
            [W_pack[:, 2 * H + (h0 + l) * D:2 * H + (h0 + l + 1) * D]
             for l in range(HC)], axis=1)                 # [H, 512]
        wh, wl = _q8hl(wv_f, S_W)
        wvv_h = wh.reshape(KC, 128, 2, 256).transpose(1, 2, 0, 3)
        wvv_l = wl.reshape(KC, 128, 2, 256).transpose(1, 2, 0, 3)
        wv_np = np.empty((128, 2, KC, 2, 256), ml_dtypes.float8_e4m3)
        wv_np[:, :, :, 0, :] = wvv_l
        wv_np[:, :, :, 1, :] = wvv_h
        wv_np = np.ascontiguousarray(wv_np.reshape(128, -1))

        wo_f = W_o[h0 * D:(h0 + HC) * D, :]               # [512, H]
        wh, wl = _q8hl(wo_f, S_W)
        wov_h = wh.reshape(HC, 128, H).transpose(1, 0, 2)
        wov_l = wl.reshape(HC, 128, H).transpose(1, 0, 2)
        wo_np = np.empty((128, HC, 2, H), ml_dtypes.float8_e4m3)
        wo_np[:, :, 0, :] = wov_h
        wo_np[:, :, 1, :] = wov_l
        wo_np = np.ascontiguousarray(wo_np.reshape(128, -1))

        in_maps.append({
            "xq": xq_np, "wqk": wqk_np, "wv": wv_np, "wo": wo_np,
            "cosT": cosT_np, "sinS": sinS_np, "maskD": maskD_np,
        })
    return in_maps


def kernel(hidden_states, W_pack, W_o, attention_mask, position_ids):
    if "nc" not in _CACHE:
        _CACHE["nc"] = _build_module()
    nc = _CACHE["nc"]
    in_maps = _host_prep(hidden_states, W_pack, W_o, attention_mask, position_ids)
    res = bass_utils.run_bass_kernel_spmd(nc, in_maps, core_ids=list(range(N_CORES)))
    out = res.results[0]["out_p"].astype(np.float32)
    for c in range(1, N_CORES):
        out += res.results[c]["out_p"]
    out *= 1.0 / (S_A * S_W)
    return out.reshape(B, S, H).astype(np.float32)


# revision 17
# speedup vs baseline: 3.3572x; 2.4571x over previous
"""Trainium2 Bass kernel for nn_Attention_60567628808865.

Dense transformer attention block (B=4, S=1024, H=4096, NH=32, D=128):
  qkv = x @ W_pack; RoPE(q, k); causal-masked softmax attention; out @ W_o.

Sharding: tensor-parallel over heads across 8 NeuronCores. Each core computes
4 heads end-to-end; the host sums the 8 partial W_o outputs (row-sharded W_o).

Precision/performance scheme (validated on host to rel_err ~2.7e-3 vs the
2e-2 gate):
  - QKV and W_o projections run in fp8(e4m3) with the DoubleRow perf mode
    (K=256 per instruction, 0.5 cycles/row) using an exact-style two-term
    decomposition: x@W ~= x_hi@W_hi + [x_hi@W_lo + x_lo@W_hi], where
    t_hi = fp8(t*s), t_lo = fp8(t*s - t_hi). Both terms accumulate into ONE
    PSUM chain (identical scale), so the epilogue is unchanged. 48 DoubleRow
    instructions replace 32 f32r instructions per [128col x 256tok] unit:
    0.75x PE cycles.
  - hi/lo operands are slot-interleaved in a single packed tensor
    ([part, chunk, 2, free]) so the correction chain reads (hi,lo) slot pairs
    and the main chain reads (hi,hi) chunk pairs from the same SBUF bytes.
  - Attention is causal-aware: score/PV/denominator work is emitted only for
    the 20/32 key-tile x query-block units on or below the diagonal; the two
    diagonal tiles per query block are masked multiplicatively with a
    host-built exp(mask) pattern (asserted causal). q/k/v round-trip DRAM in
    bf16; scores/PV matmuls run in bf16 (same PE rate as f32r, half the DMA).
  - Softmax is unnormalized; denominators come from a ones-vector matmul
    accumulated in PSUM, broadcast back via a K=1 matmul; attention output is
    quantized to fp8 hi/lo pairs on the fly for the W_o DoubleRow chain.
  - Output partials are stored bf16; the host sum applies the global descale.
"""
import numpy as np

import concourse.bass as bass  # noqa: F401
import concourse.tile as tile
from contextlib import ExitStack
from concourse import bacc, mybir
from concourse import bass_utils

F32 = mybir.dt.float32
F32R = mybir.dt.float32r
BF16 = mybir.dt.bfloat16
F8 = mybir.dt.float8e4
AF = mybir.ActivationFunctionType
ALU = mybir.AluOpType
DR = mybir.MatmulPerfMode.DoubleRow

B, S, H, NH = 4, 1024, 4096, 32
D = H // NH          # 128
T = B * S            # 4096 tokens
N_CORES = 8
HC = NH // N_CORES   # 4 heads per core
SCALE = float(1.0 / np.sqrt(D))
ROPE_BASE = 10000.0

TB = 256             # phase-1 token block
NTB = T // TB        # 16
KC = H // 128        # 32 fp8 k-chunks of 128 features
S_X = 32.0           # x quant scale
S_W = 2048.0         # W_pack / W_o quant scale
S_A = 32.0           # attention-output quant scale
DESCALE = 1.0 / (S_X * S_W)

_CACHE = {}


def _build_module(phases=("p1", "p2")):
    nc = bacc.Bacc("TRN2", target_bir_lowering=False, debug=False,
                   num_devices=N_CORES)

    # packed fp8 inputs (see _host_prep for layouts)
    xq = nc.dram_tensor("xq", [128, NTB * KC * 2 * TB], F8, kind="ExternalInput").ap()
    wqk = nc.dram_tensor("wqk", [128, 8 * KC * 2 * 128], F8, kind="ExternalInput").ap()
    wv = nc.dram_tensor("wv", [128, 2 * KC * 2 * 256], F8, kind="ExternalInput").ap()
    wo = nc.dram_tensor("wo", [128, HC * 2 * H], F8, kind="ExternalInput").ap()
    cosT = nc.dram_tensor("cosT", [128, T], F32, kind="ExternalInput").ap()
    sinS = nc.dram_tensor("sinS", [128, T], F32, kind="ExternalInput").ap()
    maskD = nc.dram_tensor("maskD", [128, 512], BF16, kind="ExternalInput").ap()
    out_p = nc.dram_tensor("out_p", [T, H], BF16, kind="ExternalOutput").ap()

    import ml_dtypes
    ones128 = nc.inline_tensor(
        np.ones((128, 1), ml_dtypes.bfloat16), "ones128").ap()
    onesS = nc.inline_tensor(
        np.full((1, 128), S_A, np.float32), "onesS").ap().bitcast(F32R)

    with tile.TileContext(nc) as tc, \
         nc.allow_low_precision(reason="fp8/bf16 matmuls; verified vs reference"):
        with ExitStack() as octx:
            dram = octx.enter_context(tc.tile_pool(name="dram", bufs=1, space="DRAM"))
            cpool = octx.enter_context(tc.tile_pool(name="consts", bufs=1))
            # DRAM scratch: qkT rows ordered [q0,k0,q1,k1,q2,k2,q3,k3] x d
            qkT_d = dram.tile([8 * 128, T], BF16)
            v_d = dram.tile([T, HC * 128], BF16)

            o128 = cpool.tile([128, 1], BF16)
            nc.sync.dma_start(o128[:], ones128[:])
            oS = cpool.tile([1, 128], F32R)
            nc.sync.dma_start(oS[:], onesS[:])
            mask_t = cpool.tile([128, 512], BF16)
            nc.sync.dma_start(mask_t[:], maskD[:])

            # phase-2 tiles prefetched during phase 1 (wo_a has no deps; the
            # first head's kq/vt depend on the tb0-3 scratch stores)
            wopool = octx.enter_context(tc.tile_pool(name="p2wo", bufs=1))
            kqpool = octx.enter_context(tc.tile_pool(name="p2kq", bufs=2))
            vtpool = octx.enter_context(tc.tile_pool(name="p2vt", bufs=2))
            _wo_a = [None]
            _first_kv = [None]

            def load_kv(b, l):
                bs = b * S
                kq = kqpool.tile([128, 2, S], BF16, tag="kq")
                nc.sync.dma_start(
                    kq[:],
                    qkT_d[l * 256:(l + 1) * 256, bs:bs + S]
                        .rearrange("(j p) t -> p j t", p=128))
                vt = vtpool.tile([128, 8, 128], BF16, tag="vt")
                nc.sync.dma_start(
                    vt[:],
                    v_d[bs:bs + S, l * 128:(l + 1) * 128]
                        .rearrange("(kt p) d -> p kt d", p=128))
                return kq, vt

            def prefetch_wo():
                # W_o resident: [128, h(4), j(2), c(4096)]; j=0 -> hi, 1 -> lo
                wo_a = wopool.tile([128, HC, 2, H], F8, tag="wo")
                for h in range(HC):
                    nc.sync.dma_start(
                        wo_a[:, h],
                        wo[:, h * 2 * H:(h + 1) * 2 * H]
                            .rearrange("p (j c) -> p j c", j=2))
                _wo_a[0] = wo_a

            def prefetch_kv():
                _first_kv[0] = load_kv(0, 0)

            # ---------------- Phase 1: QKV projection (fp8 DoubleRow) -------
            if "p1" in phases:
              with ExitStack() as ctx:
                wpool = ctx.enter_context(tc.tile_pool(name="p1w", bufs=1))
                xpool = ctx.enter_context(tc.tile_pool(name="p1x", bufs=2))
                opool = ctx.enter_context(tc.tile_pool(name="p1o", bufs=2))
                cspool = ctx.enter_context(tc.tile_pool(name="p1cs", bufs=2))
                rpool = ctx.enter_context(tc.tile_pool(name="p1rope", bufs=3))
                pqk = ctx.enter_context(tc.tile_pool(name="p1pqk", bufs=4, space="PSUM"))
                pv = ctx.enter_context(tc.tile_pool(name="p1pv", bufs=2, space="PSUM"))

                def load_tb(tb):
                    t0 = tb * TB
                    # x pack [128, kk(32), j(2), t(256)]; j=0 -> x_hi, j=1 -> x_lo
                    xall = xpool.tile([128, KC, 2, TB], F8, tag="x")
                    nc.sync.dma_start(
                        xall[:],
                        xq[:, tb * 16384:(tb + 1) * 16384]
                            .rearrange("p (kk j t) -> p kk j t", kk=KC, j=2))
                    cos_tb = cspool.tile([128, TB], F32, tag="cos")
                    nc.sync.dma_start(cos_tb[:], cosT[:, t0:t0 + TB])
                    sin_tb = cspool.tile([128, TB], F32, tag="sin")
                    nc.sync.dma_start(sin_tb[:], sinS[:, t0:t0 + TB])
                    return xall, cos_tb, sin_tb

                # tb0 inputs first (first chain needs x + wqk ct0 only), then
                # resident weights: wqk [128, ct(8), kk(32), j(2), c(128)],
                # wv [128, ct(2), kk(32), j(2), c(256)]; j=0 -> W_lo, j=1 -> W_hi
                tb0_inputs = load_tb(0)
                wqk_a = wpool.tile([128, 8, KC, 2, 128], F8, tag="wqk")
                wv_a = wpool.tile([128, 2, KC, 2, 256], F8, tag="wv")
                for ct in range(8):
                    nc.sync.dma_start(
                        wqk_a[:, ct],
                        wqk[:, ct * 8192:(ct + 1) * 8192]
                            .rearrange("p (kk j c) -> p kk j c", kk=KC, j=2))
                for cv in range(2):
                    nc.sync.dma_start(
                        wv_a[:, cv],
                        wv[:, cv * 16384:(cv + 1) * 16384]
                            .rearrange("p (kk j c) -> p kk j c", kk=KC, j=2))

                for tb in range(NTB):
                    t0 = tb * TB
                    if tb == 0:
                        xall, cos_tb, sin_tb = tb0_inputs
                    else:
                        xall, cos_tb, sin_tb = load_tb(tb)
                    if tb == 1:
                        prefetch_wo()
                    elif tb == 4:
                        prefetch_kv()

                    qs_all = opool.tile([128, 8, TB], BF16, tag="qs")
                    for i in range(8):
                        ps = pqk.tile([128, TB], F32, tag="qk")
                        for c in range(16):
                            nc.tensor.matmul(
                                ps[:], wqk_a[:, i, 2 * c:2 * c + 2, 1, :],
                                xall[:, 2 * c:2 * c + 2, 0, :],
                                start=(c == 0), stop=False, perf_mode=DR)
                        for kk in range(KC):
                            nc.tensor.matmul(
                                ps[:], wqk_a[:, i, kk, :, :],
                                xall[:, kk, :, :],
                                start=False, stop=(kk == KC - 1), perf_mode=DR)
                        # RoPE epilogue (psum scale folded into cos/sin tables)
                        rot = rpool.tile([128, TB], F32, tag="rot")
                        nc.scalar.copy(rot[0:64, :], ps[64:128, :])
                        nc.vector.tensor_copy(rot[64:128, :], ps[0:64, :])
                        m1_ = rpool.tile([128, TB], F32, tag="m1")
                        nc.vector.tensor_tensor(m1_[:], ps[:], cos_tb[:], op=ALU.mult)
                        m2_ = rpool.tile([128, TB], F32, tag="m2")
                        nc.vector.tensor_tensor(m2_[:], rot[:], sin_tb[:], op=ALU.mult)
                        nc.vector.tensor_tensor(qs_all[:, i, :], m1_[:], m2_[:],
                                                op=ALU.add)
                    nc.sync.dma_start(
                        qkT_d[:, t0:t0 + TB].rearrange("(i p) t -> p i t", p=128),
                        qs_all[:])

                    vs_all = opool.tile([128, 2, 2, 256], BF16, tag="vs")
                    for th in range(2):
                        for ch in range(2):
                            ps = pv.tile([128, 256], F32, tag="v")
                            for c in range(16):
                                nc.tensor.matmul(
                                    ps[:],
                                    xall[:, 2 * c:2 * c + 2, 0,
                                         th * 128:(th + 1) * 128],
                                    wv_a[:, ch, 2 * c:2 * c + 2, 1, :],
                                    start=(c == 0), stop=False, perf_mode=DR)
                            for kk in range(KC):
                                nc.tensor.matmul(
                                    ps[:],
                                    xall[:, kk, :, th * 128:(th + 1) * 128],
                                    wv_a[:, ch, kk, :, :],
                                    start=False, stop=(kk == KC - 1), perf_mode=DR)
                            nc.scalar.activation(vs_all[:, th, ch, :], ps[:],
                                                 AF.Copy, scale=DESCALE)
                    nc.sync.dma_start(
                        v_d[t0:t0 + TB, :]
                            .rearrange("(th p) (ch c) -> p th ch c", p=128, ch=2),
                        vs_all[:])

            # ---------------- Phase 2: attention + W_o ----------------------
            if "p2" in phases:
              with ExitStack() as ctx:
                apool = ctx.enter_context(tc.tile_pool(name="p2a", bufs=2))
                efpool = ctx.enter_context(tc.tile_pool(name="p2ef", bufs=14))
                tpool = ctx.enter_context(tc.tile_pool(name="p2t", bufs=3))
                rpool2 = ctx.enter_context(tc.tile_pool(name="p2rd", bufs=3))
                opool = ctx.enter_context(tc.tile_pool(name="p2o", bufs=2))
                ps_s = ctx.enter_context(tc.tile_pool(name="p2ps", bufs=3, space="PSUM"))
                ps_av = ctx.enter_context(tc.tile_pool(name="p2pav", bufs=2, space="PSUM"))
                ps_d = ctx.enter_context(tc.tile_pool(name="p2pd", bufs=1, space="PSUM"))
                ps_o = ctx.enter_context(tc.tile_pool(name="p2po", bufs=2, space="PSUM"))

                if _wo_a[0] is None:     # p2-only debug build
                    prefetch_wo()
                    prefetch_kv()
                wo_a = _wo_a[0]
                for b in range(B):
                    bs = b * S
                    # attn pack [128, lh(2), l(4), t(1024)]; lh=0 -> lo, 1 -> hi
                    apack = apool.tile([128, 2, HC, S], F8, tag="apack")
                    for l in range(HC):
                        if b == 0 and l == 0:
                            kq, vt = _first_kv[0]
                        else:
                            kq, vt = load_kv(b, l)

                        pending = [None]
                        psd_l = ps_d.tile([1, 512], F32, tag="dbc")
                        all_efs = {}

                        def emit_scores(qb):
                            u = 2 * qb + 2
                            q_sl = kq[:, 0, qb * 256:(qb + 1) * 256]
                            efs = [None] * u
                            # diagonal pair first: its exp+mask latency hides
                            # behind the remaining pairs' matmuls
                            for g in [qb] + list(range(qb)):
                                pss = ps_s.tile([128, 512], F32, tag="s")
                                for sHalf in range(2):
                                    mt = 2 * g + sHalf
                                    nc.tensor.matmul(
                                        pss[:, sHalf * 256:(sHalf + 1) * 256],
                                        kq[:, 1, mt * 128:(mt + 1) * 128],
                                        q_sl, start=True, stop=True)
                                ef = efpool.tile([128, 512], BF16, tag="ef")
                                nc.scalar.activation(ef[:], pss[:], AF.Exp,
                                                     scale=SCALE)
                                if g == qb:  # diagonal: multiplicative mask
                                    efm = efpool.tile([128, 512], BF16, tag="efm")
                                    nc.vector.tensor_tensor(efm[:], ef[:],
                                                            mask_t[:], op=ALU.mult)
                                    ef = efm
                                efs[2 * g] = ef[:, 0:256]
                                efs[2 * g + 1] = ef[:, 256:512]
                            all_efs[qb] = efs

                        def emit_pv(qb):
                            u = 2 * qb + 2
                            efs = all_efs.pop(qb)
                            # masked diagonal units last in the chains
                            order = list(range(2 * qb)) + [2 * qb, 2 * qb + 1]
                            if pending[0] is not None:
                                pending[0]()
                                pending[0] = None
                            psav = ps_av.tile([128, 256], F32, tag="av")
                            for n, mt in enumerate(order):
                                nc.tensor.matmul(
                                    psav[:], vt[:, mt, :], efs[mt],
                                    start=(n == 0), stop=(n == u - 1))
                            psd = psd_l[:, (qb % 2) * 256:(qb % 2 + 1) * 256]
                            for n, mt in enumerate(order):
                                nc.tensor.matmul(
                                    psd, o128[:], efs[mt],
                                    start=(n == 0), stop=(n == u - 1))
                            rd = rpool2.tile([1, 256], F32R, tag="rd")
                            nc.vector.reciprocal(rd, psd)

                            def make_epilogue(qb=qb, psav=psav, rd=rd):
                                def emit():
                                    psbc = ps_o.tile([128, 256], F32, tag="o")
                                    nc.tensor.matmul(psbc[:], oS[:], rd[:],
                                                     start=True, stop=True)
                                    t_ = tpool.tile([128, 256], F32, tag="t")
                                    nc.vector.tensor_tensor(t_[:], psav[:],
                                                            psbc[:], op=ALU.mult)
                                    q0 = qb * 256
                                    hi = apack[:, 1, l, q0:q0 + 256]
                                    nc.vector.tensor_copy(hi, t_[:])
                                    nc.vector.tensor_tensor(
                                        apack[:, 0, l, q0:q0 + 256], t_[:], hi,
                                        op=ALU.subtract)
                                return emit
                            pending[0] = make_epilogue()

                        # scores run two query-blocks ahead of PV/denominator
                        emit_scores(0)
                        emit_scores(1)
                        emit_scores(2)
                        emit_pv(0)
                        emit_scores(3)
                        emit_pv(1)
                        emit_pv(2)
                        emit_pv(3)
                        pending[0]()

                    # W_o projection for batch b (fp8 DoubleRow main+corr);
                    # two 256-col chains per PSUM bank, one wide copy each
                    for m in range(8):
                        osb = opool.tile([128, 8, 512], BF16, tag="osb")
                        msl = slice(m * 128, (m + 1) * 128)
                        for pair in range(8):
                            pso = ps_o.tile([128, 512], F32, tag="o")
                            for half in range(2):
                                csl = slice((2 * pair + half) * 256,
                                            (2 * pair + half + 1) * 256)
                                po = pso[:, half * 256:(half + 1) * 256]
                                for c in range(2):
                                    nc.tensor.matmul(
                                        po, apack[:, 1, 2 * c:2 * c + 2, msl],
                                        wo_a[:, 2 * c:2 * c + 2, 0, csl],
                                        start=(c == 0), stop=False, perf_mode=DR)
                                for h in range(HC):
                                    nc.tensor.matmul(
                                        po, apack[:, :, h, msl],
                                        wo_a[:, h, :, csl],
                                        start=False, stop=(h == HC - 1),
                                        perf_mode=DR)
                            if pair % 2 == 0:
                                nc.vector.tensor_copy(osb[:, pair, :], pso[:])
                            else:
                                nc.scalar.copy(osb[:, pair, :], pso[:])
                        nc.sync.dma_start(
                            out_p[bs + m * 128:bs + (m + 1) * 128, :],
                            osb[:].rearrange("p nc c -> p (nc c)"))
    nc.compile()
    return nc


def _q8hl(a, scale):
    """Quantize to fp8 e4m3 hi/lo pair at a shared scale."""
    import ml_dtypes
    hi = (a * scale).astype(ml_dtypes.float8_e4m3)
    lo = ((a * scale) - hi.astype(np.float32)).astype(ml_dtypes.float8_e4m3)
    return hi, lo


def _host_prep(hidden_states, W_pack, W_o, attention_mask, position_ids):
    import ml_dtypes
    x = np.asarray(hidden_states, dtype=np.float32).reshape(T, H)
    W_pack = np.asarray(W_pack, dtype=np.float32)
    W_o = np.asarray(W_o, dtype=np.float32)
    mask = np.asarray(attention_mask, dtype=np.float32)
    pos = np.asarray(position_ids)

    # causal structure is hardcoded in the kernel; verify it holds
    m0 = mask[0, 0]
    iu = np.triu_indices(S, 1)
    assert (m0[iu] < -1e8).all() and (np.tril(m0) == 0).all(), \
        "kernel requires the standard causal mask"

    # x pack: [128p, tb, kk, j(hi,lo), t] -> flat [128, NTB*KC*2*TB]
    xh, xl = _q8hl(x, S_X)
    xv_h = xh.reshape(NTB, TB, KC, 128).transpose(3, 0, 2, 1)
    xv_l = xl.reshape(NTB, TB, KC, 128).transpose(3, 0, 2, 1)
    xq_np = np.empty((128, NTB, KC, 2, TB), ml_dtypes.float8_e4m3)
    xq_np[:, :, :, 0, :] = xv_h
    xq_np[:, :, :, 1, :] = xv_l
    xq_np = np.ascontiguousarray(xq_np.reshape(128, -1))

    # rope tables with the fp8 descale folded in; rotate-half sign in sinS
    inv = 1.0 / (ROPE_BASE ** (np.arange(0, D, 2, dtype=np.float64) / D))
    inv = np.concatenate([inv, inv])
    ang = pos.astype(np.float64).reshape(T)[None, :] * inv[:, None]   # [D, T]
    cosT_np = np.ascontiguousarray((np.cos(ang) * DESCALE).astype(np.float32))
    sinT = (np.sin(ang) * DESCALE).astype(np.float32)
    sinS_np = sinT.copy()
    sinS_np[:64] = -sinT[:64]
    sinS_np = np.ascontiguousarray(sinS_np)

    # diagonal exp-mask patterns [128p(key), s(2)*256(query)] bf16
    em = np.exp(m0)
    maskD_np = np.empty((128, 2, 256), ml_dtypes.bfloat16)
    maskD_np[:, 0, :] = em[0:256, 0:128].T       # offset 0 pattern
    maskD_np[:, 1, :] = em[0:256, 128:256].T     # offset 128 pattern
    maskD_np = np.ascontiguousarray(maskD_np.reshape(128, 512))

    in_maps = []
    for core in range(N_CORES):
        h0 = core * HC
        # wqk cols ordered [q0,k0,q1,k1,q2,k2,q3,k3] per head slice
        cols = []
        for l in range(HC):
            cols.append(W_pack[:, (h0 + l) * D:(h0 + l + 1) * D])
            cols.append(W_pack[:, H + (h0 + l) * D:H + (h0 + l + 1) * D])
        wqk_f = np.concatenate(cols, axis=1)              # [H, 1024]
        wh, wl = _q8hl(wqk_f, S_W)
        wv_h = wh.reshape(KC, 128, 8, 128).transpose(1, 2, 0, 3)
        wv_l = wl.reshape(KC, 128, 8, 128).transpose(1, 2, 0, 3)
        wqk_np = np.empty((128, 8, KC, 2, 128), ml_dtypes.float8_e4m3)
        wqk_np[:, :, :, 0, :] = wv_l
        wqk_np[:, :, :, 1, :] = wv_h
        wqk_np = np.ascontiguousarray(wqk_np.reshape(128, -1))

        wv_f = np.concatenate(
            [W_pack[:, 2 * H + (h0 + l) * D:2 * H + (h0 + l + 1) * D]
             for l in range(HC)], axis=1)                 # [H, 512]
        wh, wl = _q8hl(wv_f, S_W)
        wvv_h = wh.reshape(KC, 128, 2, 256).transpose(1, 2, 0, 3)
        wvv_l = wl.reshape(KC, 128, 2, 256).transpose(1, 2, 0, 3)
        wv_np = np.empty((128, 2, KC, 2, 256), ml_dtypes.float8_e4m3)
        wv_np[:, :, :, 0, :] = wvv_l
        wv_np[:, :, :, 1, :] = wvv_h
        wv_np = np.ascontiguousarray(wv_np.reshape(128, -1))

        wo_f = W_o[h0 * D:(h0 + HC) * D, :]               # [512, H]
        wh, wl = _q8hl(wo_f, S_W)
        wov_h = wh.reshape(HC, 128, H).transpose(1, 0, 2)
        wov_l = wl.reshape(HC, 128, H).transpose(1, 0, 2)
        wo_np = np.empty((128, HC, 2, H), ml_dtypes.float8_e4m3)
        wo_np[:, :, 0, :] = wov_h
        wo_np[:, :, 1, :] = wov_l
        wo_np = np.ascontiguousarray(wo_np.reshape(128, -1))

        in_maps.append({
            "xq": xq_np, "wqk": wqk_np, "wv": wv_np, "wo": wo_np,
            "cosT": cosT_np, "sinS": sinS_np, "maskD": maskD_np,
        })
    return in_maps


def kernel(hidden_states, W_pack, W_o, attention_mask, position_ids):
    if "nc" not in _CACHE:
        _CACHE["nc"] = _build_module()
    nc = _CACHE["nc"]
    in_maps = _host_prep(hidden_states, W_pack, W_o, attention_mask, position_ids)
    res = bass_utils.run_bass_kernel_spmd(nc, in_maps, core_ids=list(range(N_CORES)))
    out = res.results[0]["out_p"].astype(np.float32)
    for c in range(1, N_CORES):
        out += res.results[c]["out_p"]
    out *= 1.0 / (S_A * S_W)
    return out.reshape(B, S, H).astype(np.float32)
